# revision 38
# baseline (speedup 1.0000x reference)
"""Trainium2 Bass kernel for nn_ClipOTLoss (CLIP-style OT/Sinkhorn loss).

Computes, for full inputs features[B,D], prototypes[K,D], logits[B,K]:
    w = normalize(prototypes, axis=1)
    sims = features @ w.T / TEMPERATURE
    soft_code = sinkhorn(sims)            (3 iters, eps=0.7)
    loss = -mean_b sum_k soft_code * log_softmax(logits)

Distribution: data-parallel over B across 8 NeuronCores; prototypes
replicated; the Sinkhorn row-marginal (sum over B per prototype k)
is a 16KB AllReduce per iteration.  Per-core partial losses are summed
on the host (no final AllReduce).

Design notes (v4):
- Host stages transposed inputs: features.T (bf16), prototypes.T (fp8,
  entries are ~N(0,1) so e4m3 holds them directly), logits (bf16).
  Zero PE transposes; all reference FLOPs run on device.
- Prototype normalization in place on the fp8 tile: squares (split
  between DVE and ACT), PE ones-matvec, rn = exp(-0.5*ln(norm2) +
  ln(32)), broadcast multiply applied per K-half so the main matmul's
  first half starts while the second half is still normalizing.
- Main matmul runs fp8 DoubleRow; one LSE exp op interleaves after
  each (h,c) unit so ScalarE retires the logits row-sums during this
  phase.
- Sinkhorn: per-k ratios applied eagerly to E in place; per-b factors
  stay lazily in the matvec stationary vector.  Column-sum
  accumulations are split between DVE (fused STT) and ACT (Copy with
  accum_out into dead WN8 storage) to balance engines.
- Dead matmuls on a constant source pad the PE queue through each
  AllReduce + multiply window, keeping the HAM clock warm so the next
  matvec runs at full rate.
- A dummy max-AllReduce triggered once this core's logits land (folded
  into beta as x*0+1) absorbs cross-core DMA skew before the first
  real AllReduce.
- log_softmax never materialized: loss_b = LSE_b - dot'_b/s_b with 1/s
  applied after accumulation.
"""

import os
import sys

import numpy as np

sys.path.insert(0, "/opt/trn_rl_repo")

import concourse.bass as bass  # noqa: E402
import concourse.bacc as bacc  # noqa: E402
import concourse.tile as tile  # noqa: E402
import concourse.mybir as mybir  # noqa: E402

F32 = mybir.dt.float32
BF16 = mybir.dt.bfloat16
FP8 = mybir.dt.float8e4
AF = mybir.ActivationFunctionType
ALU = mybir.AluOpType
PM = mybir.MatmulPerfMode

TEMPERATURE = 0.01
EPSILON = 0.7
NUM_ITERS = 3
TINY = 1e-8

P = 128
NSLICE = 512
SF_W = 32.0
SF_F = 256.0
N_STT = 3  # mult-pass chunks running fused on DVE (rest TT + ACT accum)
N_DOT_STT = 4  # tail dot chunks on DVE STT (rest TT + ACT accum)
W_AR = 40  # PE warm matmuls covering the AllReduce window
W_MULT = 80  # PE warm matmuls covering the multiply window


def build_nc(B_loc=1024, K=4096, D=1024, n_cores=8):
    NB = B_loc // P
    ND = D // P
    NK = K // P
    KH = K // 2
    exp_scale = 1.0 / (TEMPERATURE * EPSILON * SF_W * SF_F)
    r_marg = 1.0 / K
    c_marg = 1.0 / (B_loc * n_cores)
    loss_scale = 1.0 / (B_loc * n_cores)
    rg = [list(range(n_cores))]

    nc = bacc.Bacc(None, target_bir_lowering=False, debug=False)

    fT = nc.declare_dram_parameter("fT", [D, B_loc], BF16, isOutput=False)
    wT8 = nc.declare_dram_parameter("wT8", [D, K], FP8, isOutput=False)
    lg_d = nc.declare_dram_parameter("lg", [B_loc, K], BF16, isOutput=False)
    out_ext = nc.declare_dram_parameter("out", [1], F32, isOutput=True)

    m_in_d = [nc.dram_tensor(f"cc_m_in{i}", [K], F32) for i in range(NUM_ITERS)]
    m_out_d = [
        nc.dram_tensor(f"cc_m_out{i}", [K], F32, addr_space="Shared")
        for i in range(NUM_ITERS)
    ]
    d_in_d = nc.dram_tensor("cc_d_in", [P], F32)
    d_out_d = nc.dram_tensor("cc_d_out", [P], F32, addr_space="Shared")

    with tile.TileContext(nc) as tc:
        with (
            tc.tile_pool(name="single", bufs=1) as single,
            tc.tile_pool(name="stage", bufs=2) as stg,
            tc.tile_pool(name="rows", bufs=1) as rows,
            tc.tile_pool(name="big", bufs=1) as bigp,
            tc.tile_pool(name="ps", bufs=2, space="PSUM") as psp,
        ):
            smf = single.tile([P, 400], F32, tag="smf")
            smb = single.tile([P, 1312], BF16, tag="smb")

            class _Cols:
                def __init__(self, t):
                    self.t, self.off = t, 0

                def take(self, np_, nf):
                    ap = self.t[:np_, self.off : self.off + nf]
                    self.off += nf
                    return ap

            cf, cb = _Cols(smf), _Cols(smb)
            ones_col_f = cf.take(P, 1)
            cs_fl = cf.take(P, NB * 2)
            se_fl = cf.take(P, NB)
            se2 = cf.take(P, NB)
            lse = cf.take(P, NB)
            cs0 = cf.take(P, NB)
            beta = cf.take(P, NB)
            tmpb = cf.take(P, NB)
            vp_fl = cf.take(P, NB * NUM_ITERS)
            rs = cf.take(P, NB)
            dot_fl = cf.take(P, NB)
            dotn = cf.take(P, NB)
            losses = cf.take(P, NB)
            lcol = cf.take(P, 1)
            mg_sb = cf.take(NK, P)
            rt_f = cf.take(NK, P)
            loss_sb = cf.take(1, 8)
            dcol = cf.take(P, 1)
            gate = cf.take(P, 1)
            lnw_col = cf.take(P, 1)

            ones_col_bf = cb.take(P, 1)
            beta_bf = cb.take(P, NB)
            rt_bf = cb.take(NK, P)
            ones_row_bf = cb.take(1, P)
            dead_bf = cb.take(P, NSLICE)
            dead2_bf = cb.take(P, NSLICE)

            nc.vector.memset(ones_col_f, 1.0)
            nc.vector.memset(ones_col_bf, 1.0)
            nc.vector.memset(ones_row_bf, 1.0)
            nc.vector.memset(lnw_col, float(np.log(SF_W)))
            nc.vector.memset(dead_bf, 1.0)
            # GpSimd throughput probe (no consumers, runs off critical
            # path; read its duration from the trace)
            nc.gpsimd.tensor_tensor(
                out=dead2_bf, in0=dead_bf, in1=dead_bf, op=ALU.mult
            )

            # ---- persistent big tensors ----
            E = bigp.tile([P, NB, K], BF16, tag="E")
            LG = bigp.tile([P, NB, K], BF16, tag="LG")
            WN8 = bigp.tile([P, ND, K], FP8, tag="WN8")
            F8 = bigp.tile([P, ND, B_loc], FP8, tag="F8")
            RBC = bigp.tile([P, K], BF16, tag="RBC")

            # scratch views over storage that is dead at time of use
            sq_regs = [E[:, 0, :], E[:, 2, :], E[:, 4, :], E[:, 6, :]]
            rnbc_scr = E[:, 1, :]
            act_scr = WN8[:, :, :].rearrange("p a b -> p (a b)").bitcast(BF16)

            # =========================================================
            # Input DMAs.  Prototypes first (they gate normalization),
            # per d-chunk so squares start as chunks land.
            # =========================================================
            wT8v = wT8.rearrange("(j p) k -> p j k", p=P)
            for j in range(ND):
                nc.sync.dma_start(out=WN8[:, j, :], in_=wT8v[:, j, :])
            ft_tiles = []
            for j in range(ND):
                ft_in = stg.tile([P, KH], BF16, tag="stage")
                nc.sync.dma_start(
                    out=ft_in[:, :B_loc], in_=fT[j * P : (j + 1) * P, :]
                )
                ft_tiles.append(ft_in)
            for c in range(NB):
                nc.sync.dma_start(out=LG[:, c, :], in_=lg_d[c * P : (c + 1) * P, :])

            # dummy skew-absorbing AllReduce (see module docstring)
            nc.vector.tensor_copy(out=dcol, in_=LG[:, NB - 1, 0:1])
            nc.sync.dma_start(out=d_in_d[:], in_=dcol)
            nc.gpsimd.collective_compute(
                "AllReduce",
                ALU.max,
                replica_groups=rg,
                ins=[d_in_d[:]],
                outs=[d_out_d[:]],
            )
            nc.sync.dma_start(
                out=gate, in_=d_out_d[:].rearrange("(a b) -> a b", a=P)
            )
            nc.vector.tensor_scalar(
                out=gate, in0=gate, scalar1=0.0, scalar2=1.0,
                op0=ALU.mult, op1=ALU.add,
            )

            # =========================================================
            # Prototype normalization in place on WN8.
            # Features cast first on ACT (so staging slots recycle and
            # the DMA queue never stalls); squares split DVE/ACT.
            # =========================================================
            for j in range(ND):
                nc.scalar.activation(
                    out=F8[:, j, :], in_=ft_tiles[j][:, :B_loc], func=AF.Copy,
                    scale=SF_F,
                )
            for j in range(4):
                nc.vector.tensor_tensor(
                    out=sq_regs[j % 2], in0=WN8[:, j, :], in1=WN8[:, j, :],
                    op=ALU.mult,
                )
            for j in range(4, ND):
                nc.scalar.activation(
                    out=sq_regs[2 + j % 2], in_=WN8[:, j, :], func=AF.Square
                )
            # norm2 = ones^T @ squares (PE, accumulated over chunks)
            nv0 = psp.tile([1, KH], F32, tag="ps")
            nv1 = psp.tile([1, KH], F32, tag="ps")
            nv = [nv0, nv1]
            for j in range(ND):
                sq = sq_regs[j % 2] if j < 4 else sq_regs[2 + j % 2]
                for half in range(2):
                    for n in range(KH // NSLICE):
                        nc.tensor.matmul(
                            nv[half][:1, n * NSLICE : (n + 1) * NSLICE],
                            ones_col_bf[:, :1],
                            sq[:, half * KH + n * NSLICE : half * KH + (n + 1) * NSLICE],
                            start=(j == 0),
                            stop=(j == ND - 1),
                        )
            rn_row = rows.tile([1, K], BF16, tag="row")
            for half in range(2):
                nc.scalar.activation(
                    out=nv[half][:1, :], in_=nv[half][:1, :], func=AF.Ln
                )
                nc.scalar.activation(
                    out=nv[half][:1, :], in_=nv[half][:1, :], func=AF.Exp,
                    scale=-0.5, bias=lnw_col[:1, :1],
                )
                nc.vector.tensor_copy(
                    out=rn_row[:1, half * KH : (half + 1) * KH], in_=nv[half][:1, :]
                )
            # broadcast rn across partitions, then normalize per K-half
            # (half 0 finishes first so the main matmul's h=0 block can
            # start while half 1 is still being applied)
            for g in range(2):
                rb = psp.tile([P, KH], F32, tag="ps")
                for n in range(KH // NSLICE):
                    nc.tensor.matmul(
                        rb[:, n * NSLICE : (n + 1) * NSLICE],
                        ones_row_bf[:1, :],
                        rn_row[:1, g * KH + n * NSLICE : g * KH + (n + 1) * NSLICE],
                        start=True,
                        stop=True,
                    )
                nc.vector.tensor_copy(
                    out=rnbc_scr[:, g * KH : (g + 1) * KH], in_=rb[:, :]
                )
            for g in range(2):
                for j in range(ND):
                    nc.vector.tensor_tensor(
                        out=WN8[:, j, g * KH : (g + 1) * KH],
                        in0=WN8[:, j, g * KH : (g + 1) * KH],
                        in1=rnbc_scr[:, g * KH : (g + 1) * KH],
                        op=ALU.mult,
                    )

            # =========================================================
            # Main matmul (fp8 DoubleRow) + exp.  LSE exps interleave
            # into the back half of the phase: by then every core's
            # logits have long arrived (no cross-core skew coupling),
            # and ScalarE has slack between the PSUM-drain exps.
            # =========================================================
            lse_units = [(c, q) for c in range(NB) for q in range(2)]
            lse_emitted = 0
            unit = 0
            for h in range(2):
                for c in range(NB):
                    mm_ps = psp.tile([P, KH], F32, tag="ps")
                    for j2 in range(0, ND, 2):
                        for n in range(KH // NSLICE):
                            nc.tensor.matmul(
                                mm_ps[:, n * NSLICE : (n + 1) * NSLICE],
                                F8[:, j2 : j2 + 2, c * P : (c + 1) * P],
                                WN8[
                                    :,
                                    j2 : j2 + 2,
                                    h * KH + n * NSLICE : h * KH + (n + 1) * NSLICE,
                                ],
                                start=(j2 == 0),
                                stop=(j2 == ND - 2),
                                perf_mode=PM.DoubleRow,
                            )
                    nc.scalar.activation(
                        out=E[:, c, h * KH : (h + 1) * KH],
                        in_=mm_ps[:],
                        func=AF.Exp,
                        scale=exp_scale,
                        accum_out=cs_fl[:, c * 2 + h : c * 2 + h + 1],
                    )
            # LSE exps strictly AFTER exp(E) in the ACT queue: on
            # stragglers the logits DMA lands late, and any coupling
            # of the first AllReduce's trigger to logits arrival makes
            # every core pay that skew (measured +17us on AllReduce 1).
            for lc, lq in lse_units:
                lse_scr = stg.tile([P, KH], BF16, tag="stage")
                nc.scalar.activation(
                    out=lse_scr[:, :],
                    in_=LG[:, lc, lq * KH : (lq + 1) * KH],
                    func=AF.Exp,
                    accum_out=(se_fl if lq == 0 else se2)[:, lc : lc + 1],
                )
            nc.vector.tensor_tensor(out=se_fl, in0=se_fl, in1=se2, op=ALU.add)

            # beta0 = (1/colsum0) * gate
            cs_pair = cs_fl.rearrange("p (c two) -> p c two", two=2)
            nc.vector.tensor_tensor(
                out=cs0, in0=cs_pair[:, :, 0], in1=cs_pair[:, :, 1], op=ALU.add
            )
            nc.vector.reciprocal(out=beta, in_=cs0)
            nc.vector.tensor_scalar(
                out=beta, in0=beta, scalar1=gate, scalar2=None, op0=ALU.mult
            )
            nc.vector.tensor_copy(out=beta_bf, in_=beta)

            # =========================================================
            # Sinkhorn iterations.  Iteration i's multiply pass feeds
            # iteration i+1's matvec chunk-by-chunk: beta[b] for chunk
            # c depends only on chunk c's column-sum, so beta updates
            # per chunk-pair and the matvec follows two chunks behind
            # the multiplies instead of waiting for the whole pass.
            # =========================================================

            def emit_matvec(it, mv0, mv1, cs):
                """matvec MM groups for chunks cs into held psums."""
                for c in cs:
                    for half, mv in ((0, mv0), (1, mv1)):
                        for n in range(KH // NSLICE):
                            nc.tensor.matmul(
                                mv[
                                    :1, n * NSLICE : (n + 1) * NSLICE
                                ],
                                beta_bf[:, c : c + 1],
                                E[
                                    :,
                                    c,
                                    half * KH
                                    + n * NSLICE : half * KH
                                    + (n + 1) * NSLICE,
                                ],
                                start=(c == 0),
                                stop=(c == NB - 1),
                            )

            def emit_ar(it, mv0, mv1):
                """m rows out of PSUM, bounce to DRAM, AllReduce."""
                for half, mv in ((0, mv0), (1, mv1)):
                    mr = rows.tile([1, KH], F32, tag="mrow")
                    if half == 0:
                        nc.vector.tensor_copy(out=mr[:1, :], in_=mv[:1, :])
                    else:
                        nc.scalar.copy(out=mr[:1, :], in_=mv[:1, :])
                    nc.sync.dma_start(
                        out=m_in_d[it][half * KH : (half + 1) * KH], in_=mr[:1, :]
                    )
                nc.gpsimd.collective_compute(
                    "AllReduce",
                    ALU.add,
                    replica_groups=rg,
                    ins=[m_in_d[it][:]],
                    outs=[m_out_d[it][:]],
                )

            # iteration 0's matvec stands alone (needs beta0 from the
            # exp colsums, which need the whole matmul phase anyway)
            mv0 = psp.tile([1, KH], F32, tag="ps")
            mv1 = psp.tile([1, KH], F32, tag="ps")
            emit_matvec(0, mv0, mv1, range(NB))
            emit_ar(0, mv0, mv1)

            for it in range(NUM_ITERS):
                last = it == NUM_ITERS - 1
                nc.sync.dma_start(
                    out=mg_sb[:], in_=m_out_d[it][:].rearrange("(a b) -> a b", a=NK)
                )
                # ratio = 1 / (m/r + TINY/r)
                nc.vector.tensor_scalar(
                    out=rt_f[:], in0=mg_sb[:], scalar1=1.0 / r_marg,
                    scalar2=TINY / r_marg, op0=ALU.mult, op1=ALU.add,
                )
                with nc.allow_low_precision(reason="ratio rounds to bf16 anyway"):
                    nc.vector.reciprocal(out=rt_bf[:], in_=rt_f[:])
                rt1 = rows.tile([1, K], BF16, tag="row")
                nc.sync.dma_start(out=rt1[:1, :], in_=rt_bf[:])
                for g in range(2):
                    rb_ps = psp.tile([P, KH], F32, tag="ps")
                    for n in range(KH // NSLICE):
                        nc.tensor.matmul(
                            rb_ps[:, n * NSLICE : (n + 1) * NSLICE],
                            ones_row_bf[:1, :],
                            rt1[:1, g * KH + n * NSLICE : g * KH + (n + 1) * NSLICE],
                            start=True,
                            stop=True,
                        )
                    if g == 0:
                        nc.scalar.copy(
                            out=RBC[:, g * KH : (g + 1) * KH], in_=rb_ps[:, :]
                        )
                    else:
                        nc.vector.tensor_copy(
                            out=RBC[:, g * KH : (g + 1) * KH], in_=rb_ps[:, :]
                        )
                # ---- E *= ratio_bc (in place) with col-sums -> vp.
                # Chunk 0 runs on GpSimd (otherwise idle); early chunks
                # TT + ACT accum, late chunks fused STT on DVE.
                vp_c = lambda c: vp_fl[:, it * NB + c : it * NB + c + 1]
                if not last:
                    nmv0 = psp.tile([1, KH], F32, tag="ps")
                    nmv1 = psp.tile([1, KH], F32, tag="ps")
                for c in range(NB):
                    if not last and c % 2 == 1:
                        # odd chunks: fused STT on DVE (immediate vp,
                        # completes each beta pair fastest)
                        nc.vector.scalar_tensor_tensor(
                            out=E[:, c, :],
                            in0=E[:, c, :],
                            scalar=1.0,
                            in1=RBC[:, :],
                            op0=ALU.mult,
                            op1=ALU.mult,
                            accum_out=vp_c(c),
                        )
                    elif not last:
                        nc.vector.tensor_tensor(
                            out=E[:, c, :], in0=E[:, c, :], in1=RBC[:, :],
                            op=ALU.mult,
                        )
                        nc.scalar.activation(
                            out=act_scr[:, (c % 2) * K : (c % 2 + 1) * K],
                            in_=E[:, c, :],
                            func=AF.Copy,
                            accum_out=vp_c(c),
                        )
                    else:
                        nc.vector.tensor_tensor(
                            out=E[:, c, :], in0=E[:, c, :], in1=RBC[:, :],
                            op=ALU.mult,
                        )
                        nc.scalar.activation(
                            out=act_scr[:, (c % 2) * K : (c % 2 + 1) * K],
                            in_=E[:, c, :],
                            func=AF.Copy,
                            accum_out=vp_c(c),
                        )
                    if last:
                        # dot'[b] = sum_k Q*logits (1/s applied later);
                        # everything writes scratch so ACT's s-accum
                        # reads of E are never blocked.
                        if c < N_DOT_STT:
                            nc.vector.scalar_tensor_tensor(
                                out=act_scr[:, (2 + c % 2) * K : (3 + c % 2) * K],
                                in0=E[:, c, :],
                                scalar=1.0,
                                in1=LG[:, c, :],
                                op0=ALU.mult,
                                op1=ALU.mult,
                                accum_out=dot_fl[:, c : c + 1],
                            )
                        else:
                            nc.vector.tensor_tensor(
                                out=act_scr[:, (2 + c % 2) * K : (3 + c % 2) * K],
                                in0=E[:, c, :],
                                in1=LG[:, c, :],
                                op=ALU.mult,
                            )
                            nc.scalar.activation(
                                out=act_scr[:, (2 + c % 2) * K : (3 + c % 2) * K],
                                in_=act_scr[:, (2 + c % 2) * K : (3 + c % 2) * K],
                                func=AF.Copy,
                                accum_out=dot_fl[:, c : c + 1],
                            )
                    if not last and c % 2 == 1:
                        # beta[:, c-1:c+1] *= c_marg/(beta*vp + TINY),
                        # then this pair's matvec for the NEXT iteration
                        pr = slice(c - 1, c + 1)
                        vp_pr = vp_fl[:, it * NB + c - 1 : it * NB + c + 1]
                        nc.vector.tensor_tensor(
                            out=tmpb[:, pr], in0=beta[:, pr], in1=vp_pr,
                            op=ALU.mult,
                        )
                        nc.vector.tensor_scalar(
                            out=tmpb[:, pr], in0=tmpb[:, pr], scalar1=TINY,
                            scalar2=None, op0=ALU.add,
                        )
                        nc.vector.reciprocal(out=tmpb[:, pr], in_=tmpb[:, pr])
                        nc.vector.tensor_scalar(
                            out=tmpb[:, pr], in0=tmpb[:, pr], scalar1=c_marg,
                            scalar2=None, op0=ALU.mult,
                        )
                        nc.vector.tensor_tensor(
                            out=beta[:, pr], in0=beta[:, pr], in1=tmpb[:, pr],
                            op=ALU.mult,
                        )
                        nc.vector.tensor_copy(
                            out=beta_bf[:, pr], in_=beta[:, pr]
                        )
                        emit_matvec(it + 1, nmv0, nmv1, [c - 1, c])
                if not last:
                    emit_ar(it + 1, nmv0, nmv1)

            # =========================================================
            # Loss: loss_b = LSE_b - dot'_b / s_b,  s = vp3
            # =========================================================
            nc.vector.reciprocal(
                out=rs, in_=vp_fl[:, (NUM_ITERS - 1) * NB : NUM_ITERS * NB]
            )
            nc.scalar.activation(out=lse, in_=se_fl, func=AF.Ln)
            nc.vector.tensor_tensor(out=dotn, in0=dot_fl, in1=rs, op=ALU.mult)
            nc.vector.tensor_tensor(out=losses, in0=lse, in1=dotn, op=ALU.subtract)
            nc.vector.tensor_reduce(
                out=lcol, in_=losses, axis=mybir.AxisListType.X, op=ALU.add
            )
            lp_ps = psp.tile([1, 1], F32, tag="ps")
            nc.tensor.matmul(
                lp_ps[:1, :1], ones_col_f[:, :1], lcol[:, :1], start=True, stop=True
            )
            nc.vector.tensor_scalar(
                out=loss_sb[:1, 0:1], in0=lp_ps[:1, :1], scalar1=loss_scale,
                scalar2=None, op0=ALU.mult,
            )
            nc.sync.dma_start(out=out_ext[:], in_=loss_sb[:1, 0:1])

    nc.compile()
    return nc


LAST_RESULT = None


def kernel(features, prototypes, logits):
    from concourse.bass_utils import run_bass_kernel_spmd
    import ml_dtypes

    global LAST_RESULT
    n_cores = 8
    B, D = features.shape
    K = prototypes.shape[0]
    B_loc = B // n_cores

    nc = build_nc(B_loc=B_loc, K=K, D=D, n_cores=n_cores)

    bf16 = ml_dtypes.bfloat16
    f8 = ml_dtypes.float8_e4m3
    # host staging: shard + transpose + dtype cast (layout/precision
    # prep only; all reference FLOPs run on device)
    wT8 = np.ascontiguousarray(prototypes.T).astype(f8)
    in_maps = []
    for i in range(n_cores):
        fsl = features[i * B_loc : (i + 1) * B_loc]
        in_maps.append(
            {
                "fT": np.ascontiguousarray(fsl.T).astype(bf16),
                "wT8": wT8,
                "lg": logits[i * B_loc : (i + 1) * B_loc].astype(bf16),
            }
        )
    res = run_bass_kernel_spmd(
        nc,
        in_maps,
        list(range(n_cores)),
        trace=bool(os.environ.get("CLIP_OT_TRACE")),
    )
    LAST_RESULT = res
    total = 0.0
    for i in range(n_cores):
        total += float(np.asarray(res.results[i]["out"]).reshape(-1)[0])
    return np.float32(total)


# revision 39
# speedup vs baseline: 1.0319x; 1.0319x over previous
"""Trainium2 Bass kernel for nn_ClipOTLoss (CLIP-style OT/Sinkhorn loss).

Computes, for full inputs features[B,D], prototypes[K,D], logits[B,K]:
    w = normalize(prototypes, axis=1)
    sims = features @ w.T / TEMPERATURE
    soft_code = sinkhorn(sims)            (3 iters, eps=0.7)
    loss = -mean_b sum_k soft_code * log_softmax(logits)

Distribution: data-parallel over B across 8 NeuronCores; prototypes
replicated; the Sinkhorn row-marginal (sum over B per prototype k)
is a 16KB AllReduce per iteration.  Per-core partial losses are summed
on the host (no final AllReduce).

Design notes (v4):
- Host stages transposed inputs: features.T (bf16), prototypes.T (fp8,
  entries are ~N(0,1) so e4m3 holds them directly), logits (bf16).
  Zero PE transposes; all reference FLOPs run on device.
- Prototype normalization in place on the fp8 tile: squares (split
  between DVE and ACT), PE ones-matvec, rn = exp(-0.5*ln(norm2) +
  ln(32)), broadcast multiply applied per K-half so the main matmul's
  first half starts while the second half is still normalizing.
- Main matmul runs fp8 DoubleRow; one LSE exp op interleaves after
  each (h,c) unit so ScalarE retires the logits row-sums during this
  phase.
- Sinkhorn: per-k ratios applied eagerly to E in place; per-b factors
  stay lazily in the matvec stationary vector.  Column-sum
  accumulations are split between DVE (fused STT) and ACT (Copy with
  accum_out into dead WN8 storage) to balance engines.
- Dead matmuls on a constant source pad the PE queue through each
  AllReduce + multiply window, keeping the HAM clock warm so the next
  matvec runs at full rate.
- A dummy max-AllReduce triggered once this core's logits land (folded
  into beta as x*0+1) absorbs cross-core DMA skew before the first
  real AllReduce.
- log_softmax never materialized: loss_b = LSE_b - dot'_b/s_b with 1/s
  applied after accumulation.
"""

import os
import sys

import numpy as np

sys.path.insert(0, "/opt/trn_rl_repo")

import concourse.bass as bass  # noqa: E402
import concourse.bacc as bacc  # noqa: E402
import concourse.tile as tile  # noqa: E402
import concourse.mybir as mybir  # noqa: E402

F32 = mybir.dt.float32
BF16 = mybir.dt.bfloat16
FP8 = mybir.dt.float8e4
AF = mybir.ActivationFunctionType
ALU = mybir.AluOpType
PM = mybir.MatmulPerfMode

TEMPERATURE = 0.01
EPSILON = 0.7
NUM_ITERS = 3
TINY = 1e-8

P = 128
NSLICE = 512
SF_W = 32.0
SF_F = 256.0
N_STT = 3  # mult-pass chunks running fused on DVE (rest TT + ACT accum)
N_DOT_STT = 4  # tail dot chunks on DVE STT (rest TT + ACT accum)
W_AR = 40  # PE warm matmuls covering the AllReduce window
W_MULT = 80  # PE warm matmuls covering the multiply window


def build_nc(B_loc=1024, K=4096, D=1024, n_cores=8):
    NB = B_loc // P
    ND = D // P
    NK = K // P
    KH = K // 2
    exp_scale = 1.0 / (TEMPERATURE * EPSILON * SF_W * SF_F)
    r_marg = 1.0 / K
    c_marg = 1.0 / (B_loc * n_cores)
    loss_scale = 1.0 / (B_loc * n_cores)
    rg = [list(range(n_cores))]

    nc = bacc.Bacc(None, target_bir_lowering=False, debug=False)

    fT = nc.declare_dram_parameter("fT", [D, B_loc], BF16, isOutput=False)
    wT8 = nc.declare_dram_parameter("wT8", [D, K], FP8, isOutput=False)
    lg_d = nc.declare_dram_parameter("lg", [B_loc, K], BF16, isOutput=False)
    out_ext = nc.declare_dram_parameter("out", [1], F32, isOutput=True)

    m_in_d = [nc.dram_tensor(f"cc_m_in{i}", [K], F32) for i in range(NUM_ITERS)]
    m_out_d = [
        nc.dram_tensor(f"cc_m_out{i}", [K], F32, addr_space="Shared")
        for i in range(NUM_ITERS)
    ]
    d_in_d = nc.dram_tensor("cc_d_in", [P], F32)
    d_out_d = nc.dram_tensor("cc_d_out", [P], F32, addr_space="Shared")

    with tile.TileContext(nc) as tc:
        with (
            tc.tile_pool(name="single", bufs=1) as single,
            tc.tile_pool(name="stage", bufs=2) as stg,
            tc.tile_pool(name="rows", bufs=1) as rows,
            tc.tile_pool(name="big", bufs=1) as bigp,
            tc.tile_pool(name="ps", bufs=2, space="PSUM") as psp,
        ):
            smf = single.tile([P, 400], F32, tag="smf")
            smb = single.tile([P, 1312], BF16, tag="smb")

            class _Cols:
                def __init__(self, t):
                    self.t, self.off = t, 0

                def take(self, np_, nf):
                    ap = self.t[:np_, self.off : self.off + nf]
                    self.off += nf
                    return ap

            cf, cb = _Cols(smf), _Cols(smb)
            ones_col_f = cf.take(P, 1)
            cs_fl = cf.take(P, NB * 2)
            se_fl = cf.take(P, NB)
            se2 = cf.take(P, NB)
            lse = cf.take(P, NB)
            cs0 = cf.take(P, NB)
            beta = cf.take(P, NB)
            tmpb = cf.take(P, NB)
            vp_fl = cf.take(P, NB * NUM_ITERS)
            rs = cf.take(P, NB)
            dot_fl = cf.take(P, NB)
            dotn = cf.take(P, NB)
            losses = cf.take(P, NB)
            lcol = cf.take(P, 1)
            mg_sb = cf.take(NK, P)
            rt_f = cf.take(NK, P)
            loss_sb = cf.take(1, 8)
            dcol = cf.take(P, 1)
            gate = cf.take(P, 1)
            lnw_col = cf.take(P, 1)

            ones_col_bf = cb.take(P, 1)
            beta_bf = cb.take(P, NB)
            rt_bf = cb.take(NK, P)
            ones_row_bf = cb.take(1, P)
            dead_bf = cb.take(P, NSLICE)
            dead2_bf = cb.take(P, NSLICE)

            nc.vector.memset(ones_col_f, 1.0)
            nc.vector.memset(ones_col_bf, 1.0)
            nc.vector.memset(ones_row_bf, 1.0)
            nc.vector.memset(lnw_col, float(np.log(SF_W)))
            nc.vector.memset(dead_bf, 1.0)
            # GpSimd throughput probe (no consumers, runs off critical
            # path; read its duration from the trace)
            nc.gpsimd.tensor_tensor(
                out=dead2_bf, in0=dead_bf, in1=dead_bf, op=ALU.mult
            )

            # ---- persistent big tensors ----
            E = bigp.tile([P, NB, K], BF16, tag="E")
            LG = bigp.tile([P, NB, K], BF16, tag="LG")
            WN8 = bigp.tile([P, ND, K], FP8, tag="WN8")
            F8 = bigp.tile([P, ND, B_loc], FP8, tag="F8")
            RBC = bigp.tile([P, K], BF16, tag="RBC")

            # scratch views over storage that is dead at time of use
            sq_regs = [E[:, 0, :], E[:, 2, :], E[:, 4, :], E[:, 6, :]]
            rnbc_scr = E[:, 1, :]
            act_scr = WN8[:, :, :].rearrange("p a b -> p (a b)").bitcast(BF16)

            # =========================================================
            # Input DMAs.  Prototypes first (they gate normalization),
            # per d-chunk so squares start as chunks land.
            # =========================================================
            wT8v = wT8.rearrange("(j p) k -> p j k", p=P)
            for j in range(ND):
                nc.sync.dma_start(out=WN8[:, j, :], in_=wT8v[:, j, :])
            ft_tiles = []
            for j in range(ND):
                ft_in = stg.tile([P, KH], BF16, tag="stage")
                nc.sync.dma_start(
                    out=ft_in[:, :B_loc], in_=fT[j * P : (j + 1) * P, :]
                )
                ft_tiles.append(ft_in)
            for c in range(NB):
                nc.sync.dma_start(out=LG[:, c, :], in_=lg_d[c * P : (c + 1) * P, :])

            # dummy skew-absorbing AllReduce (see module docstring)
            nc.vector.tensor_copy(out=dcol, in_=LG[:, NB - 1, 0:1])
            nc.sync.dma_start(out=d_in_d[:], in_=dcol)
            nc.gpsimd.collective_compute(
                "AllReduce",
                ALU.max,
                replica_groups=rg,
                ins=[d_in_d[:]],
                outs=[d_out_d[:]],
            )
            nc.sync.dma_start(
                out=gate, in_=d_out_d[:].rearrange("(a b) -> a b", a=P)
            )
            nc.vector.tensor_scalar(
                out=gate, in0=gate, scalar1=0.0, scalar2=1.0,
                op0=ALU.mult, op1=ALU.add,
            )

            # =========================================================
            # Prototype normalization in place on WN8.
            # Features cast first on ACT (so staging slots recycle and
            # the DMA queue never stalls); squares split DVE/ACT.
            # =========================================================
            for j in range(ND):
                nc.scalar.activation(
                    out=F8[:, j, :], in_=ft_tiles[j][:, :B_loc], func=AF.Copy,
                    scale=SF_F,
                )
            for j in range(4):
                nc.vector.tensor_tensor(
                    out=sq_regs[j % 2], in0=WN8[:, j, :], in1=WN8[:, j, :],
                    op=ALU.mult,
                )
            for j in range(4, ND):
                nc.scalar.activation(
                    out=sq_regs[2 + j % 2], in_=WN8[:, j, :], func=AF.Square
                )
            # norm2 = ones^T @ squares (PE, accumulated over chunks)
            nv0 = psp.tile([1, KH], F32, tag="ps")
            nv1 = psp.tile([1, KH], F32, tag="ps")
            nv = [nv0, nv1]
            for j in range(ND):
                sq = sq_regs[j % 2] if j < 4 else sq_regs[2 + j % 2]
                for half in range(2):
                    for n in range(KH // NSLICE):
                        nc.tensor.matmul(
                            nv[half][:1, n * NSLICE : (n + 1) * NSLICE],
                            ones_col_bf[:, :1],
                            sq[:, half * KH + n * NSLICE : half * KH + (n + 1) * NSLICE],
                            start=(j == 0),
                            stop=(j == ND - 1),
                        )
            rn_row = rows.tile([1, K], BF16, tag="row")
            for half in range(2):
                nc.scalar.activation(
                    out=nv[half][:1, :], in_=nv[half][:1, :], func=AF.Ln
                )
                nc.scalar.activation(
                    out=nv[half][:1, :], in_=nv[half][:1, :], func=AF.Exp,
                    scale=-0.5, bias=lnw_col[:1, :1],
                )
                nc.vector.tensor_copy(
                    out=rn_row[:1, half * KH : (half + 1) * KH], in_=nv[half][:1, :]
                )
            # broadcast rn across partitions, then normalize per K-half
            # (half 0 finishes first so the main matmul's h=0 block can
            # start while half 1 is still being applied)
            for g in range(2):
                rb = psp.tile([P, KH], F32, tag="ps")
                for n in range(KH // NSLICE):
                    nc.tensor.matmul(
                        rb[:, n * NSLICE : (n + 1) * NSLICE],
                        ones_row_bf[:1, :],
                        rn_row[:1, g * KH + n * NSLICE : g * KH + (n + 1) * NSLICE],
                        start=True,
                        stop=True,
                    )
                nc.vector.tensor_copy(
                    out=rnbc_scr[:, g * KH : (g + 1) * KH], in_=rb[:, :]
                )
            for g in range(2):
                for j in range(ND):
                    nc.vector.tensor_tensor(
                        out=WN8[:, j, g * KH : (g + 1) * KH],
                        in0=WN8[:, j, g * KH : (g + 1) * KH],
                        in1=rnbc_scr[:, g * KH : (g + 1) * KH],
                        op=ALU.mult,
                    )

            # =========================================================
            # Main matmul (fp8 DoubleRow) + exp.  LSE exps interleave
            # into the back half of the phase: by then every core's
            # logits have long arrived (no cross-core skew coupling),
            # and ScalarE has slack between the PSUM-drain exps.
            # =========================================================
            lse_units = [(c, q) for c in range(NB) for q in range(2)]
            lse_emitted = 0
            unit = 0
            for h in range(2):
                for c in range(NB):
                    mm_ps = psp.tile([P, KH], F32, tag="ps")
                    for j2 in range(0, ND, 2):
                        for n in range(KH // NSLICE):
                            nc.tensor.matmul(
                                mm_ps[:, n * NSLICE : (n + 1) * NSLICE],
                                F8[:, j2 : j2 + 2, c * P : (c + 1) * P],
                                WN8[
                                    :,
                                    j2 : j2 + 2,
                                    h * KH + n * NSLICE : h * KH + (n + 1) * NSLICE,
                                ],
                                start=(j2 == 0),
                                stop=(j2 == ND - 2),
                                perf_mode=PM.DoubleRow,
                            )
                    nc.scalar.activation(
                        out=E[:, c, h * KH : (h + 1) * KH],
                        in_=mm_ps[:],
                        func=AF.Exp,
                        scale=exp_scale,
                        accum_out=cs_fl[:, c * 2 + h : c * 2 + h + 1],
                    )
            # LSE exps strictly AFTER exp(E) in the ACT queue: on
            # stragglers the logits DMA lands late, and any coupling
            # of the first AllReduce's trigger to logits arrival makes
            # every core pay that skew (measured +17us on AllReduce 1).
            for lc, lq in lse_units:
                lse_scr = stg.tile([P, KH], BF16, tag="stage")
                nc.scalar.activation(
                    out=lse_scr[:, :],
                    in_=LG[:, lc, lq * KH : (lq + 1) * KH],
                    func=AF.Exp,
                    accum_out=(se_fl if lq == 0 else se2)[:, lc : lc + 1],
                )
            nc.vector.tensor_tensor(out=se_fl, in0=se_fl, in1=se2, op=ALU.add)

            # beta0 = (1/colsum0) * gate
            cs_pair = cs_fl.rearrange("p (c two) -> p c two", two=2)
            nc.vector.tensor_tensor(
                out=cs0, in0=cs_pair[:, :, 0], in1=cs_pair[:, :, 1], op=ALU.add
            )
            nc.vector.reciprocal(out=beta, in_=cs0)
            nc.vector.tensor_scalar(
                out=beta, in0=beta, scalar1=gate, scalar2=None, op0=ALU.mult
            )
            nc.vector.tensor_copy(out=beta_bf, in_=beta)

            # =========================================================
            # Sinkhorn iterations.  Iteration i's multiply pass feeds
            # iteration i+1's matvec chunk-by-chunk: beta[b] for chunk
            # c depends only on chunk c's column-sum, so beta updates
            # per chunk-pair and the matvec follows two chunks behind
            # the multiplies instead of waiting for the whole pass.
            # =========================================================

            def emit_matvec(it, mv0, mv1, cs):
                """matvec MM groups for chunks cs into held psums."""
                for c in cs:
                    for half, mv in ((0, mv0), (1, mv1)):
                        for n in range(KH // NSLICE):
                            nc.tensor.matmul(
                                mv[
                                    :1, n * NSLICE : (n + 1) * NSLICE
                                ],
                                beta_bf[:, c : c + 1],
                                E[
                                    :,
                                    c,
                                    half * KH
                                    + n * NSLICE : half * KH
                                    + (n + 1) * NSLICE,
                                ],
                                start=(c == 0),
                                stop=(c == NB - 1),
                            )

            def emit_ar(it, mv0, mv1):
                """m rows out of PSUM, bounce to DRAM, AllReduce."""
                for half, mv in ((0, mv0), (1, mv1)):
                    mr = rows.tile([1, KH], F32, tag="mrow")
                    if half == 0:
                        nc.vector.tensor_copy(out=mr[:1, :], in_=mv[:1, :])
                    else:
                        nc.scalar.copy(out=mr[:1, :], in_=mv[:1, :])
                    nc.sync.dma_start(
                        out=m_in_d[it][half * KH : (half + 1) * KH], in_=mr[:1, :]
                    )
                nc.gpsimd.collective_compute(
                    "AllReduce",
                    ALU.add,
                    replica_groups=rg,
                    ins=[m_in_d[it][:]],
                    outs=[m_out_d[it][:]],
                )

            # iteration 0's matvec stands alone (needs beta0 from the
            # exp colsums, which need the whole matmul phase anyway)
            mv0 = psp.tile([1, KH], F32, tag="ps")
            mv1 = psp.tile([1, KH], F32, tag="ps")
            emit_matvec(0, mv0, mv1, range(NB))
            emit_ar(0, mv0, mv1)

            for it in range(NUM_ITERS):
                last = it == NUM_ITERS - 1
                nc.sync.dma_start(
                    out=mg_sb[:], in_=m_out_d[it][:].rearrange("(a b) -> a b", a=NK)
                )
                # ratio = 1 / (m/r + TINY/r)
                nc.vector.tensor_scalar(
                    out=rt_f[:], in0=mg_sb[:], scalar1=1.0 / r_marg,
                    scalar2=TINY / r_marg, op0=ALU.mult, op1=ALU.add,
                )
                with nc.allow_low_precision(reason="ratio rounds to bf16 anyway"):
                    nc.vector.reciprocal(out=rt_bf[:], in_=rt_f[:])
                rt1 = rows.tile([1, K], BF16, tag="row")
                nc.sync.dma_start(out=rt1[:1, :], in_=rt_bf[:])
                for g in range(2):
                    rb_ps = psp.tile([P, KH], F32, tag="ps")
                    for n in range(KH // NSLICE):
                        nc.tensor.matmul(
                            rb_ps[:, n * NSLICE : (n + 1) * NSLICE],
                            ones_row_bf[:1, :],
                            rt1[:1, g * KH + n * NSLICE : g * KH + (n + 1) * NSLICE],
                            start=True,
                            stop=True,
                        )
                    if g == 0 and it > 0:
                        # (iteration 0's ACT queue is still draining
                        # LSE exps; keep its copies on DVE)
                        nc.scalar.copy(
                            out=RBC[:, g * KH : (g + 1) * KH], in_=rb_ps[:, :]
                        )
                    else:
                        nc.vector.tensor_copy(
                            out=RBC[:, g * KH : (g + 1) * KH], in_=rb_ps[:, :]
                        )
                # ---- E *= ratio_bc (in place) with col-sums -> vp.
                # Chunk 0 runs on GpSimd (otherwise idle); early chunks
                # TT + ACT accum, late chunks fused STT on DVE.
                vp_c = lambda c: vp_fl[:, it * NB + c : it * NB + c + 1]
                if not last:
                    nmv0 = psp.tile([1, KH], F32, tag="ps")
                    nmv1 = psp.tile([1, KH], F32, tag="ps")
                for c in range(NB):
                    if not last and c % 2 == 1:
                        # odd chunks: fused STT on DVE (immediate vp,
                        # completes each beta pair fastest)
                        nc.vector.scalar_tensor_tensor(
                            out=E[:, c, :],
                            in0=E[:, c, :],
                            scalar=1.0,
                            in1=RBC[:, :],
                            op0=ALU.mult,
                            op1=ALU.mult,
                            accum_out=vp_c(c),
                        )
                    elif not last:
                        nc.vector.tensor_tensor(
                            out=E[:, c, :], in0=E[:, c, :], in1=RBC[:, :],
                            op=ALU.mult,
                        )
                        nc.scalar.activation(
                            out=act_scr[:, (c % 2) * K : (c % 2 + 1) * K],
                            in_=E[:, c, :],
                            func=AF.Copy,
                            accum_out=vp_c(c),
                        )
                    else:
                        nc.vector.tensor_tensor(
                            out=E[:, c, :], in0=E[:, c, :], in1=RBC[:, :],
                            op=ALU.mult,
                        )
                        nc.scalar.activation(
                            out=act_scr[:, (c % 2) * K : (c % 2 + 1) * K],
                            in_=E[:, c, :],
                            func=AF.Copy,
                            accum_out=vp_c(c),
                        )
                    if last:
                        # dot'[b] = sum_k Q*logits (1/s applied later);
                        # everything writes scratch so ACT's s-accum
                        # reads of E are never blocked.
                        if c < N_DOT_STT:
                            nc.vector.scalar_tensor_tensor(
                                out=act_scr[:, (2 + c % 2) * K : (3 + c % 2) * K],
                                in0=E[:, c, :],
                                scalar=1.0,
                                in1=LG[:, c, :],
                                op0=ALU.mult,
                                op1=ALU.mult,
                                accum_out=dot_fl[:, c : c + 1],
                            )
                        else:
                            nc.vector.tensor_tensor(
                                out=act_scr[:, (2 + c % 2) * K : (3 + c % 2) * K],
                                in0=E[:, c, :],
                                in1=LG[:, c, :],
                                op=ALU.mult,
                            )
                            nc.scalar.activation(
                                out=act_scr[:, (2 + c % 2) * K : (3 + c % 2) * K],
                                in_=act_scr[:, (2 + c % 2) * K : (3 + c % 2) * K],
                                func=AF.Copy,
                                accum_out=dot_fl[:, c : c + 1],
                            )
                    if not last and c % 2 == 1:
                        # beta[:, c-1:c+1] *= c_marg/(beta*vp + TINY),
                        # then this pair's matvec for the NEXT iteration
                        pr = slice(c - 1, c + 1)
                        vp_pr = vp_fl[:, it * NB + c - 1 : it * NB + c + 1]
                        nc.vector.tensor_tensor(
                            out=tmpb[:, pr], in0=beta[:, pr], in1=vp_pr,
                            op=ALU.mult,
                        )
                        nc.vector.tensor_scalar(
                            out=tmpb[:, pr], in0=tmpb[:, pr], scalar1=TINY,
                            scalar2=None, op0=ALU.add,
                        )
                        nc.vector.reciprocal(out=tmpb[:, pr], in_=tmpb[:, pr])
                        nc.vector.tensor_scalar(
                            out=tmpb[:, pr], in0=tmpb[:, pr], scalar1=c_marg,
                            scalar2=None, op0=ALU.mult,
                        )
                        nc.vector.tensor_tensor(
                            out=beta[:, pr], in0=beta[:, pr], in1=tmpb[:, pr],
                            op=ALU.mult,
                        )
                        nc.vector.tensor_copy(
                            out=beta_bf[:, pr], in_=beta[:, pr]
                        )
                        emit_matvec(it + 1, nmv0, nmv1, [c - 1, c])
                if not last:
                    emit_ar(it + 1, nmv0, nmv1)

            # =========================================================
            # Loss: loss_b = LSE_b - dot'_b / s_b,  s = vp3
            # =========================================================
            nc.vector.reciprocal(
                out=rs, in_=vp_fl[:, (NUM_ITERS - 1) * NB : NUM_ITERS * NB]
            )
            nc.scalar.activation(out=lse, in_=se_fl, func=AF.Ln)
            nc.vector.tensor_tensor(out=dotn, in0=dot_fl, in1=rs, op=ALU.mult)
            nc.vector.tensor_tensor(out=losses, in0=lse, in1=dotn, op=ALU.subtract)
            nc.vector.tensor_reduce(
                out=lcol, in_=losses, axis=mybir.AxisListType.X, op=ALU.add
            )
            lp_ps = psp.tile([1, 1], F32, tag="ps")
            nc.tensor.matmul(
                lp_ps[:1, :1], ones_col_f[:, :1], lcol[:, :1], start=True, stop=True
            )
            nc.vector.tensor_scalar(
                out=loss_sb[:1, 0:1], in0=lp_ps[:1, :1], scalar1=loss_scale,
                scalar2=None, op0=ALU.mult,
            )
            nc.sync.dma_start(out=out_ext[:], in_=loss_sb[:1, 0:1])

    nc.compile()
    return nc


LAST_RESULT = None


def kernel(features, prototypes, logits):
    from concourse.bass_utils import run_bass_kernel_spmd
    import ml_dtypes

    global LAST_RESULT
    n_cores = 8
    B, D = features.shape
    K = prototypes.shape[0]
    B_loc = B // n_cores

    nc = build_nc(B_loc=B_loc, K=K, D=D, n_cores=n_cores)

    bf16 = ml_dtypes.bfloat16
    f8 = ml_dtypes.float8_e4m3
    # host staging: shard + transpose + dtype cast (layout/precision
    # prep only; all reference FLOPs run on device)
    wT8 = np.ascontiguousarray(prototypes.T).astype(f8)
    in_maps = []
    for i in range(n_cores):
        fsl = features[i * B_loc : (i + 1) * B_loc]
        in_maps.append(
            {
                "fT": np.ascontiguousarray(fsl.T).astype(bf16),
                "wT8": wT8,
                "lg": logits[i * B_loc : (i + 1) * B_loc].astype(bf16),
            }
        )
    res = run_bass_kernel_spmd(
        nc,
        in_maps,
        list(range(n_cores)),
        trace=bool(os.environ.get("CLIP_OT_TRACE")),
    )
    LAST_RESULT = res
    total = 0.0
    for i in range(n_cores):
        total += float(np.asarray(res.results[i]["out"]).reshape(-1)[0])
    return np.float32(total)
